# revision 16
# baseline (speedup 1.0000x reference)
"""Causal self-attention (B=2, T=2048, C=1024, H=16) on 8 trn2 NeuronCores.

Sharding: data-parallel over B (2) x tensor-parallel over head groups (4),
so each of the 8 cores handles one batch element and 4 heads end-to-end:
QKV projection (its W_attn column slice), full-T causal attention for its
4 heads, and the partial output projection (its W_proj row slice). The
host sums the 4 per-batch partials and adds biases.

Device dataflow (x^T is pre-transposed on the host):
  Q^T/K^T ([d, t] layout, f32r) and V ([t, d] bf16, 65-wide head slots
  with a ones column for rowsums) from x^T chunks
  S^T[k, q] = K^T.T @ Q^T per head (causal block-skipped + trimmed)
  P = exp(S/8) on ScalarE (bf16); diagonal-block masking on Pool engine
  y[q, 64+1] = P^T.T @ (V|1) per 128-q block: each (head, q-block)
  accumulator is a consecutive matmul group (PSUM zero-regions are
  bank-granular), ping-ponged across two PSUM banks; rowsum in col 64
  y *= 1/rowsum per partition (DVE, bf16 out), transpose [q,d]->[d,q]
  via the XBAR DMA transpose, then out[t, c] = y^T.T @ Wp accumulated
  over the 2 local 128-d chunks.
QKV for token group tg+1 is interleaved with attention for q-group tg.
"""

import ml_dtypes
import numpy as np

import concourse.bass as bass
import concourse.mybir as mybir
import concourse.tile as tile
from concourse import bacc
from concourse.bass_utils import run_bass_kernel_spmd

F32 = mybir.dt.float32
F32R = mybir.dt.float32r
BF16 = mybir.dt.bfloat16
AF = mybir.ActivationFunctionType

B, T, C, H = 2, 2048, 1024, 16
HD = C // H          # 64
NCORES = 8
CTILES = C // 128    # 8 contraction chunks
TT = T // 128        # 16 token tiles of 128
QG = T // 512        # 4 q-groups of 512


def build_nc(reps=1):
    nc = bacc.Bacc("TRN2", target_bir_lowering=False)

    xT_d = nc.declare_dram_parameter("xT_b", [C, T], F32R, isOutput=False)
    w_d = nc.declare_dram_parameter("w_l", [C, 768], F32R, isOutput=False)
    bqk_d = nc.declare_dram_parameter("b_qk", [4, 128], F32, isOutput=False)
    wp_d = nc.declare_dram_parameter("wp_l", [256, C], BF16, isOutput=False)
    out_d = nc.declare_dram_parameter("out_t", [T, C], F32, isOutput=True)
    y_d = nc.dram_tensor("y_scratch", [2, T, 128], BF16, kind="Internal")

    with tile.TileContext(nc) as tc:
        for _ in range(reps):
            with tc.tile_pool(name="persist", bufs=1) as pp, \
                 tc.tile_pool(name="ps1", bufs=1, space="PSUM") as ps1, \
                 tc.tile_pool(name="ps_s", bufs=2, space="PSUM") as ps_s, \
                 tc.tile_pool(name="pab", bufs=20) as pab_pool, \
                 tc.tile_pool(name="ysb", bufs=4) as ysb_pool, \
                 tc.tile_pool(name="osb", bufs=4) as osb_pool:
                _build_body(nc, tc, pp, ps1, ps_s, pab_pool, ysb_pool,
                            osb_pool, xT_d, w_d, bqk_d, wp_d, out_d, y_d)
    nc.compile()
    return nc


def _build_body(nc, tc, pp, ps1, ps_s, pab_pool, ysb_pool, osb_pool,
                xT_d, w_d, bqk_d, wp_d, out_d, y_d):
    # ---- constants ----
    ones_col_unused = None
    b_sb = pp.tile([128, 4], F32, tag="b_sb")
    nc.sync.dma_start(out=b_sb, in_=bqk_d.ap().rearrange("j p -> p j"))

    # ---- persistent activations ----
    xT = [pp.tile([128, T], F32R, tag=f"xT{ci}", name=f"xT{ci}") for ci in range(CTILES)]
    wt = [pp.tile([128, 768], F32R, tag=f"w{ci}", name=f"w{ci}") for ci in range(CTILES)]
    wp = [pp.tile([128, C], BF16, tag=f"wp{k}", name=f"wp{k}") for k in range(2)]
    qk_pair = [pp.tile([128, T], F32R, tag=f"qkp{j}", name=f"qkp{j}") for j in range(4)]
    vt = [pp.tile([128, 260], BF16, tag=f"v{t}", name=f"v{t}") for t in range(TT)]
    for t in range(TT):
        nc.vector.memset(vt[t].rearrange("p (h c) -> p h c", c=65)[:, :, 64:65], 1.0)
    yT = [pp.tile([128, T], BF16, tag=f"yT{k}", name=f"yT{k}") for k in range(2)]
    recip_sb = pp.tile([128, 8], F32, tag="recip")

    def QT(h):
        lo = 64 * (h % 2)
        return qk_pair[h // 2][lo:lo + 64, :]

    def KT(h):
        lo = 64 * (h % 2)
        return qk_pair[2 + h // 2][lo:lo + 64, :]

    # ---- PSUM: one live accumulation group per 2KB bank (zero region) ----
    # b0: QKV/V groups (sequential) | b1/b2: yq ping-pong | b3: proj
    # b4-b7: sAB one-shot S tiles (2 banks x 2 bufs)
    qkv_ps = ps1.tile([128, 512], F32, tag="qkv")
    yq_ps = [ps1.tile([128, 512], F32, tag=f"yq{i}", name=f"yq{i}") for i in range(2)]
    prj = ps1.tile([128, 512], F32, tag="prj")

    # ---- upfront DMAs: weights, then x halves, then wp ----
    for ci in range(CTILES):
        nc.sync.dma_start(out=wt[ci], in_=w_d[128 * ci:128 * (ci + 1), :])
    for ci in range(CTILES):
        nc.sync.dma_start(out=xT[ci][:, 0:1024], in_=xT_d[128 * ci:128 * (ci + 1), 0:1024])
    for k in range(2):
        nc.sync.dma_start(out=wp[k], in_=wp_d[128 * k:128 * (k + 1), :])
    for ci in range(CTILES):
        nc.sync.dma_start(out=xT[ci][:, 1024:T], in_=xT_d[128 * ci:128 * (ci + 1), 1024:T])

    for tg in range(QG):

        # ---- QKV projection for this token group (all groups share b0) ----
        for jc in range(4):
            for ci in range(CTILES):
                nc.tensor.matmul(qkv_ps, wt[ci][:, 128 * jc:128 * (jc + 1)],
                                 xT[ci][:, 512 * tg:512 * (tg + 1)],
                                 start=(ci == 0), stop=(ci == CTILES - 1))
            nc.scalar.activation(qk_pair[jc][:, 512 * tg:512 * (tg + 1)], qkv_ps,
                                 AF.Identity, bias=b_sb[:, jc:jc + 1], scale=1.0)
        for tq in range(4):
            t = 4 * tg + tq
            for ci in range(CTILES):
                nc.tensor.matmul(qkv_ps[:, 0:256], xT[ci][:, 128 * t:128 * (t + 1)],
                                 wt[ci][:, 512:768],
                                 start=(ci == 0), stop=(ci == CTILES - 1))
            nc.vector.tensor_copy(
                vt[t].rearrange("p (h c) -> p h c", c=65)[:, :, 0:64],
                qkv_ps[:, 0:256].rearrange("p (h c) -> p h c", c=64))

        # ---- attention for q-group qi = tg (head pairs sequential) ----
        qi = tg
        nkt = 4 * qi + 4
        for hp in range(2):
            hA, hB = 2 * hp, 2 * hp + 1
            # phase A: scores + exp + causal masking; keep all P tiles live
            pabs = []
            for ki in range(nkt):
                r = ki - 4 * qi
                soff = 0 if r < 1 else (128 * r if r < 3 else 256)
                sAB = ps_s.tile([128, 1024], F32, tag="s")
                for half, h in ((0, hA), (1, hB)):
                    nc.tensor.matmul(
                        sAB[:, 512 * half + soff:512 * half + 512],
                        KT(h)[:, 128 * ki:128 * (ki + 1)],
                        QT(h)[:, 512 * qi + soff:512 * (qi + 1)],
                        start=True, stop=True)
                pAB = pab_pool.tile([128, 1024], BF16, tag="p")
                if r >= 1:
                    we = 128 * r
                    nc.scalar.activation(
                        pAB.rearrange("p (h q) -> p h q", h=2)[:, :, we:512],
                        sAB.rearrange("p (h q) -> p h q", h=2)[:, :, we:512],
                        AF.Exp, scale=1.0 / np.sqrt(HD))
                else:
                    nc.scalar.activation(pAB, sAB, AF.Exp, scale=1.0 / np.sqrt(HD))
                if r >= 0:
                    # zero the upper triangle (q < k) of the diagonal block
                    for half in range(2):
                        blk = pAB[:, 512 * half + 128 * r:512 * half + 128 * (r + 1)]
                        nc.gpsimd.affine_select(
                            out=blk, in_=blk, compare_op=mybir.AluOpType.is_ge,
                            fill=0.0, base=0, pattern=[[1, 128]], channel_multiplier=-1)
                pabs.append(pAB)
            # phase B: P^T.T @ (V|1) per (head, 128-q block), consecutive
            # matmul groups ping-ponged over two PSUM banks
            ysb = ysb_pool.tile([128, 512], BF16, tag="ysb")
            for qb in range(4):
                qg_ = 4 * qi + qb
                for half, h in ((0, hA), (1, hB)):
                    bank = yq_ps[(2 * qb + half) % 2]
                    for ki in range(qg_ + 1):
                        nc.tensor.matmul(
                            bank[:, 0:65],
                            pabs[ki][:, 512 * half + 128 * qb:512 * half + 128 * (qb + 1)],
                            vt[ki][:, 65 * h:65 * (h + 1)],
                            start=(ki == 0), stop=(ki == qg_))
                    ridx = 4 * half + qb
                    with nc.allow_low_precision(reason="softmax denom reciprocal"):
                        nc.vector.reciprocal(recip_sb[:, ridx:ridx + 1], bank[:, 64:65])
                    nc.vector.tensor_scalar_mul(
                        ysb[:, 128 * qb + 64 * half:128 * qb + 64 * (half + 1)],
                        bank[:, 0:64], recip_sb[:, ridx:ridx + 1])
            # [q, d] -> [d, q]: bounce off DRAM, XBAR-transpose on reload
            slab = y_d.ap()[hp, 512 * qi:512 * (qi + 1), :]
            nc.sync.dma_start(out=slab.rearrange("(b q) d -> q b d", q=128),
                              in_=ysb.rearrange("q (b d) -> q b d", b=4))
            nc.sync.dma_start(out=yT[hp][:, 512 * qi:512 * (qi + 1)],
                              in_=slab, transpose=True)

        # ---- output projection for this q-group ----
        for qb in range(4):
            qg_ = 4 * qi + qb
            osb = osb_pool.tile([128, C], F32, tag="osb")
            for ch in range(2):
                for k in range(2):
                    nc.tensor.matmul(prj, yT[k][:, 128 * qg_:128 * (qg_ + 1)],
                                     wp[k][:, 512 * ch:512 * (ch + 1)],
                                     start=(k == 0), stop=(k == 1))
                nc.vector.tensor_copy(osb[:, 512 * ch:512 * (ch + 1)], prj)
            nc.sync.dma_start(out=out_d[128 * qg_:128 * (qg_ + 1), :], in_=osb)


_NC = None


def _get_nc():
    global _NC
    if _NC is None:
        _NC = build_nc()
    return _NC


def kernel(x, W_attn, b_attn, W_proj, b_proj, _trace=False):
    x = np.asarray(x, dtype=np.float32)
    W_attn = np.asarray(W_attn, dtype=np.float32)
    b_attn = np.asarray(b_attn, dtype=np.float32)
    W_proj = np.asarray(W_proj, dtype=np.float32)
    b_proj = np.asarray(b_proj, dtype=np.float32)

    in_maps = []
    for core in range(NCORES):
        b, hg = divmod(core, 4)
        qs = [W_attn[:, 64 * (4 * hg + h):64 * (4 * hg + h + 1)] for h in range(4)]
        ks = [W_attn[:, C + 64 * (4 * hg + h):C + 64 * (4 * hg + h + 1)] for h in range(4)]
        vs = [W_attn[:, 2 * C + 64 * (4 * hg + h):2 * C + 64 * (4 * hg + h + 1)] for h in range(4)]
        w_l = np.concatenate(qs + ks + vs, axis=1)
        bq = [b_attn[64 * (4 * hg + h):64 * (4 * hg + h + 1)] for h in range(4)]
        bk = [b_attn[C + 64 * (4 * hg + h):C + 64 * (4 * hg + h + 1)] for h in range(4)]
        b_qk = np.stack([np.concatenate(bq[0:2]), np.concatenate(bq[2:4]),
                         np.concatenate(bk[0:2]), np.concatenate(bk[2:4])])
        wp_l = np.concatenate(
            [W_proj[64 * (4 * hg + h):64 * (4 * hg + h + 1), :] for h in range(4)], axis=0)
        in_maps.append({
            "xT_b": np.ascontiguousarray(x[b].T, dtype=np.float32),
            "w_l": np.ascontiguousarray(w_l, dtype=np.float32),
            "b_qk": np.ascontiguousarray(b_qk, dtype=np.float32),
            "wp_l": np.ascontiguousarray(wp_l).astype(ml_dtypes.bfloat16),
        })

    nc = _get_nc()
    kwargs = {}
    if _trace:
        kwargs = dict(trace=True, trace_cores=[0])
    res = run_bass_kernel_spmd(nc, in_maps, core_ids=list(range(NCORES)), **kwargs)

    # V-bias folds into the output bias because softmax rows sum to 1.
    bias_total = b_proj + b_attn[2 * C:3 * C] @ W_proj
    out = np.empty((B, T, C), dtype=np.float32)
    for b in range(B):
        acc = res.results[4 * b]["out_t"].astype(np.float32).copy()
        for hg in range(1, 4):
            acc += res.results[4 * b + hg]["out_t"]
        out[b] = acc + bias_total[None, :]
    if _trace:
        return out, res
    return out


# revision 17
# speedup vs baseline: 1.0047x; 1.0047x over previous
"""Causal self-attention (B=2, T=2048, C=1024, H=16) on 8 trn2 NeuronCores.

Sharding: data-parallel over B (2) x tensor-parallel over head groups (4),
so each of the 8 cores handles one batch element and 4 heads end-to-end:
QKV projection (its W_attn column slice), full-T causal attention for its
4 heads, and the partial output projection (its W_proj row slice). The
host sums the 4 per-batch partials and adds biases.

Device dataflow (x^T is pre-transposed on the host):
  Q^T/K^T ([d, t] layout, f32r) and V ([t, d] bf16, 65-wide head slots
  with a ones column for rowsums) from x^T chunks
  S^T[k, q] = K^T.T @ Q^T per head (causal block-skipped + trimmed)
  P = exp(S/8) on ScalarE (bf16); diagonal-block masking on Pool engine
  y[q, 64+1] = P^T.T @ (V|1) per 128-q block: each (head, q-block)
  accumulator is a consecutive matmul group (PSUM zero-regions are
  bank-granular), ping-ponged across two PSUM banks; rowsum in col 64
  y *= 1/rowsum per partition (DVE, bf16 out), transpose [q,d]->[d,q]
  via the XBAR DMA transpose, then out[t, c] = y^T.T @ Wp accumulated
  over the 2 local 128-d chunks.
QKV for token group tg+1 is interleaved with attention for q-group tg.
"""

import ml_dtypes
import numpy as np

import concourse.bass as bass
import concourse.mybir as mybir
import concourse.tile as tile
from concourse import bacc
from concourse.bass_utils import run_bass_kernel_spmd

F32 = mybir.dt.float32
F32R = mybir.dt.float32r
BF16 = mybir.dt.bfloat16
AF = mybir.ActivationFunctionType

B, T, C, H = 2, 2048, 1024, 16
HD = C // H          # 64
NCORES = 8
CTILES = C // 128    # 8 contraction chunks
TT = T // 128        # 16 token tiles of 128
QG = T // 512        # 4 q-groups of 512


def build_nc(reps=1):
    nc = bacc.Bacc("TRN2", target_bir_lowering=False)

    xT_d = nc.declare_dram_parameter("xT_b", [C, T], F32R, isOutput=False)
    w_d = nc.declare_dram_parameter("w_l", [C, 768], F32R, isOutput=False)
    bqk_d = nc.declare_dram_parameter("b_qk", [4, 128], F32, isOutput=False)
    wp_d = nc.declare_dram_parameter("wp_l", [256, C], BF16, isOutput=False)
    out_d = nc.declare_dram_parameter("out_t", [T, C], F32, isOutput=True)
    y_d = nc.dram_tensor("y_scratch", [2, T, 128], BF16, kind="Internal")

    with tile.TileContext(nc) as tc:
        for _ in range(reps):
            with tc.tile_pool(name="persist", bufs=1) as pp, \
                 tc.tile_pool(name="ps1", bufs=1, space="PSUM") as ps1, \
                 tc.tile_pool(name="ps_s", bufs=2, space="PSUM") as ps_s, \
                 tc.tile_pool(name="pab", bufs=20) as pab_pool, \
                 tc.tile_pool(name="ysb", bufs=4) as ysb_pool, \
                 tc.tile_pool(name="osb", bufs=4) as osb_pool:
                _build_body(nc, tc, pp, ps1, ps_s, pab_pool, ysb_pool,
                            osb_pool, xT_d, w_d, bqk_d, wp_d, out_d, y_d)
    nc.compile()
    return nc


def _build_body(nc, tc, pp, ps1, ps_s, pab_pool, ysb_pool, osb_pool,
                xT_d, w_d, bqk_d, wp_d, out_d, y_d):
    # ---- constants ----
    ones_col_unused = None
    b_sb = pp.tile([128, 4], F32, tag="b_sb")
    nc.sync.dma_start(out=b_sb, in_=bqk_d.ap().rearrange("j p -> p j"))

    # ---- persistent activations ----
    xT = [pp.tile([128, T], F32R, tag=f"xT{ci}", name=f"xT{ci}") for ci in range(CTILES)]
    wt = [pp.tile([128, 768], F32R, tag=f"w{ci}", name=f"w{ci}") for ci in range(CTILES)]
    wp = [pp.tile([128, C], BF16, tag=f"wp{k}", name=f"wp{k}") for k in range(2)]
    qk_pair = [pp.tile([128, T], F32R, tag=f"qkp{j}", name=f"qkp{j}") for j in range(4)]
    vt = [pp.tile([128, 260], BF16, tag=f"v{t}", name=f"v{t}") for t in range(TT)]
    for t in range(TT):
        nc.vector.memset(vt[t].rearrange("p (h c) -> p h c", c=65)[:, :, 64:65], 1.0)
    yT = [pp.tile([128, T], BF16, tag=f"yT{k}", name=f"yT{k}") for k in range(2)]
    recip_sb = pp.tile([128, 8], F32, tag="recip")

    def QT(h):
        lo = 64 * (h % 2)
        return qk_pair[h // 2][lo:lo + 64, :]

    def KT(h):
        lo = 64 * (h % 2)
        return qk_pair[2 + h // 2][lo:lo + 64, :]

    # ---- PSUM: one live accumulation group per 2KB bank (zero region) ----
    # b0: QKV/V groups (sequential) | b1/b2: yq ping-pong | b3: proj
    # b4-b7: sAB one-shot S tiles (2 banks x 2 bufs)
    qkv_ps = ps1.tile([128, 512], F32, tag="qkv")
    yq_ps = [ps1.tile([128, 512], F32, tag=f"yq{i}", name=f"yq{i}") for i in range(2)]
    prj = ps1.tile([128, 512], F32, tag="prj")

    # ---- upfront DMAs: interleave tg=0 x quarters with w chunks so the
    # first QKV accumulation streams behind the loads ----
    for ci in range(CTILES):
        nc.sync.dma_start(out=xT[ci][:, 0:512], in_=xT_d[128 * ci:128 * (ci + 1), 0:512])
        nc.sync.dma_start(out=wt[ci], in_=w_d[128 * ci:128 * (ci + 1), :])
    for k in range(2):
        nc.sync.dma_start(out=wp[k], in_=wp_d[128 * k:128 * (k + 1), :])
    for ci in range(CTILES):
        nc.sync.dma_start(out=xT[ci][:, 512:T], in_=xT_d[128 * ci:128 * (ci + 1), 512:T])

    for tg in range(QG):

        # ---- QKV projection for this token group (all groups share b0) ----
        for jc in range(4):
            for ci in range(CTILES):
                nc.tensor.matmul(qkv_ps, wt[ci][:, 128 * jc:128 * (jc + 1)],
                                 xT[ci][:, 512 * tg:512 * (tg + 1)],
                                 start=(ci == 0), stop=(ci == CTILES - 1))
            nc.scalar.activation(qk_pair[jc][:, 512 * tg:512 * (tg + 1)], qkv_ps,
                                 AF.Identity, bias=b_sb[:, jc:jc + 1], scale=1.0)
        for tq in range(4):
            t = 4 * tg + tq
            for ci in range(CTILES):
                nc.tensor.matmul(qkv_ps[:, 0:256], xT[ci][:, 128 * t:128 * (t + 1)],
                                 wt[ci][:, 512:768],
                                 start=(ci == 0), stop=(ci == CTILES - 1))
            nc.vector.tensor_copy(
                vt[t].rearrange("p (h c) -> p h c", c=65)[:, :, 0:64],
                qkv_ps[:, 0:256].rearrange("p (h c) -> p h c", c=64))

        # ---- attention for q-group qi = tg (head pairs sequential) ----
        qi = tg
        nkt = 4 * qi + 4
        for hp in range(2):
            hA, hB = 2 * hp, 2 * hp + 1
            # phase A: scores + exp + causal masking; keep all P tiles live
            pabs = []
            for ki in range(nkt):
                r = ki - 4 * qi
                soff = 0 if r < 1 else (128 * r if r < 3 else 256)
                sAB = ps_s.tile([128, 1024], F32, tag="s")
                for half, h in ((0, hA), (1, hB)):
                    nc.tensor.matmul(
                        sAB[:, 512 * half + soff:512 * half + 512],
                        KT(h)[:, 128 * ki:128 * (ki + 1)],
                        QT(h)[:, 512 * qi + soff:512 * (qi + 1)],
                        start=True, stop=True)
                pAB = pab_pool.tile([128, 1024], BF16, tag="p")
                if r >= 1:
                    we = 128 * r
                    nc.scalar.activation(
                        pAB.rearrange("p (h q) -> p h q", h=2)[:, :, we:512],
                        sAB.rearrange("p (h q) -> p h q", h=2)[:, :, we:512],
                        AF.Exp, scale=1.0 / np.sqrt(HD))
                else:
                    nc.scalar.activation(pAB, sAB, AF.Exp, scale=1.0 / np.sqrt(HD))
                if r >= 0:
                    # zero the upper triangle (q < k) of the diagonal block
                    for half in range(2):
                        blk = pAB[:, 512 * half + 128 * r:512 * half + 128 * (r + 1)]
                        nc.gpsimd.affine_select(
                            out=blk, in_=blk, compare_op=mybir.AluOpType.is_ge,
                            fill=0.0, base=0, pattern=[[1, 128]], channel_multiplier=-1)
                pabs.append(pAB)
            # phase B: P^T.T @ (V|1) per (head, 128-q block), consecutive
            # matmul groups ping-ponged over two PSUM banks
            ysb = ysb_pool.tile([128, 512], BF16, tag="ysb")
            for qb in range(4):
                qg_ = 4 * qi + qb
                for half, h in ((0, hA), (1, hB)):
                    bank = yq_ps[(2 * qb + half) % 2]
                    for ki in range(qg_ + 1):
                        nc.tensor.matmul(
                            bank[:, 0:65],
                            pabs[ki][:, 512 * half + 128 * qb:512 * half + 128 * (qb + 1)],
                            vt[ki][:, 65 * h:65 * (h + 1)],
                            start=(ki == 0), stop=(ki == qg_))
                    ridx = 4 * half + qb
                    with nc.allow_low_precision(reason="softmax denom reciprocal"):
                        nc.vector.reciprocal(recip_sb[:, ridx:ridx + 1], bank[:, 64:65])
                    nc.vector.tensor_scalar_mul(
                        ysb[:, 128 * qb + 64 * half:128 * qb + 64 * (half + 1)],
                        bank[:, 0:64], recip_sb[:, ridx:ridx + 1])
            # [q, d] -> [d, q]: bounce off DRAM, XBAR-transpose on reload
            slab = y_d.ap()[hp, 512 * qi:512 * (qi + 1), :]
            nc.sync.dma_start(out=slab.rearrange("(b q) d -> q b d", q=128),
                              in_=ysb.rearrange("q (b d) -> q b d", b=4))
            nc.sync.dma_start(out=yT[hp][:, 512 * qi:512 * (qi + 1)],
                              in_=slab, transpose=True)

        # ---- output projection for this q-group ----
        for qb in range(4):
            qg_ = 4 * qi + qb
            osb = osb_pool.tile([128, C], F32, tag="osb")
            for ch in range(2):
                for k in range(2):
                    nc.tensor.matmul(prj, yT[k][:, 128 * qg_:128 * (qg_ + 1)],
                                     wp[k][:, 512 * ch:512 * (ch + 1)],
                                     start=(k == 0), stop=(k == 1))
                nc.vector.tensor_copy(osb[:, 512 * ch:512 * (ch + 1)], prj)
            nc.sync.dma_start(out=out_d[128 * qg_:128 * (qg_ + 1), :], in_=osb)


_NC = None


def _get_nc():
    global _NC
    if _NC is None:
        _NC = build_nc()
    return _NC


def kernel(x, W_attn, b_attn, W_proj, b_proj, _trace=False):
    x = np.asarray(x, dtype=np.float32)
    W_attn = np.asarray(W_attn, dtype=np.float32)
    b_attn = np.asarray(b_attn, dtype=np.float32)
    W_proj = np.asarray(W_proj, dtype=np.float32)
    b_proj = np.asarray(b_proj, dtype=np.float32)

    in_maps = []
    for core in range(NCORES):
        b, hg = divmod(core, 4)
        qs = [W_attn[:, 64 * (4 * hg + h):64 * (4 * hg + h + 1)] for h in range(4)]
        ks = [W_attn[:, C + 64 * (4 * hg + h):C + 64 * (4 * hg + h + 1)] for h in range(4)]
        vs = [W_attn[:, 2 * C + 64 * (4 * hg + h):2 * C + 64 * (4 * hg + h + 1)] for h in range(4)]
        w_l = np.concatenate(qs + ks + vs, axis=1)
        bq = [b_attn[64 * (4 * hg + h):64 * (4 * hg + h + 1)] for h in range(4)]
        bk = [b_attn[C + 64 * (4 * hg + h):C + 64 * (4 * hg + h + 1)] for h in range(4)]
        b_qk = np.stack([np.concatenate(bq[0:2]), np.concatenate(bq[2:4]),
                         np.concatenate(bk[0:2]), np.concatenate(bk[2:4])])
        wp_l = np.concatenate(
            [W_proj[64 * (4 * hg + h):64 * (4 * hg + h + 1), :] for h in range(4)], axis=0)
        in_maps.append({
            "xT_b": np.ascontiguousarray(x[b].T, dtype=np.float32),
            "w_l": np.ascontiguousarray(w_l, dtype=np.float32),
            "b_qk": np.ascontiguousarray(b_qk, dtype=np.float32),
            "wp_l": np.ascontiguousarray(wp_l).astype(ml_dtypes.bfloat16),
        })

    nc = _get_nc()
    kwargs = {}
    if _trace:
        kwargs = dict(trace=True, trace_cores=[0])
    res = run_bass_kernel_spmd(nc, in_maps, core_ids=list(range(NCORES)), **kwargs)

    # V-bias folds into the output bias because softmax rows sum to 1.
    bias_total = b_proj + b_attn[2 * C:3 * C] @ W_proj
    out = np.empty((B, T, C), dtype=np.float32)
    for b in range(B):
        acc = res.results[4 * b]["out_t"].astype(np.float32).copy()
        for hg in range(1, 4):
            acc += res.results[4 * b + hg]["out_t"]
        out[b] = acc + bias_total[None, :]
    if _trace:
        return out, res
    return out
